# revision 14
# baseline (speedup 1.0000x reference)
"""Trainium2 Bass kernel for InteractorwoLSTM additive attention.

out[b,t,:] = alpha[b,t,:] @ h_s[b]  with
  beta[b,t,n] = W_w . tanh(h_s[b,n]@W_S + b_S + h_v[b,t]@W_V + b_V) + b_w
  alpha = masked-softmax(beta) per reference semantics.

v4 design: length-truncated slots + JIT DMA + tight ramp/tail.

Positions n >= lengths[b] never influence the output (the reference's
masked-softmax renormalization cancels them), so the tanh/add/beta work
for those positions is skipped. lengths are known on the host before
compile, so the program is built for the actual length profile:
batches sorted by length desc; slot k on core c holds sorted rank
8k+c; slot k's n-bound = L(rank 8k) (SPMD shares one program). For the
seed-0 data: bounds [30,17,12,8] -> 67 packed n-columns vs 120
(~44% less work on ACT -- the bottleneck engine at 1 elem/cycle/lane).

v4 over v3 (v3 = 59.8us, ACT busy 34.5us, ramp-to-first-tanh 16us,
tail 9us):
  - weight/input DRAM layouts are chunk-outermost so each DMA piece is
    a contiguous 1KB-per-partition run; pieces are ordered/queued so
    the first granule's deps (WS0/hsT/WV0/hvT0) land first.
  - slot-0 beta matmuls are emitted one chunk late so the PE queue
    never blocks projections behind tanh-gated work.
  - softmax chunk-sums read beta PSUM in c01/c23 halves as soon as
    each half is done -> the tail only carries the c23 copy.
  - last granule and final matmul/copy split for a shorter tail.
  - dummy tanh at the top forces the ACT table load (~2.7us) under the
    initial DMAs.
"""

import numpy as np

B, T, N = 32, 128, 30
D = 512
NCORES = 8
BPC = B // NCORES  # batch slots per core
NC = D // 128  # 4 chunks of 128 along D

_CACHE = {}


def _build(bounds):
    import concourse.bacc as bacc
    import concourse.tile as tile
    from concourse import mybir
    from concourse.masks import make_identity

    f32 = mybir.dt.float32
    bf16 = mybir.dt.bfloat16

    offs = [0]
    for b in bounds:
        offs.append(offs[-1] + b)
    P = offs[-1]
    B0 = bounds[0]

    nc = bacc.Bacc(
        "TRN2",
        target_bir_lowering=False,
        debug=False,
        enable_asserts=True,
        num_devices=NCORES,
    )

    # ---- DRAM I/O (host-prepped layouts, chunk-outermost for DMA) ----
    hvT_d = nc.dram_tensor("hvT", [BPC, 128, NC, 128], bf16, kind="ExternalInput").ap()
    hsT_d = nc.dram_tensor("hsT", [128, NC, P], bf16, kind="ExternalInput").ap()
    hs_d = nc.dram_tensor("hs", [B0, BPC, D], bf16, kind="ExternalInput").ap()
    WS_d = nc.dram_tensor("WS", [NC, 128, NC, 128], bf16, kind="ExternalInput").ap()
    WV_d = nc.dram_tensor("WV", [NC, 128, NC, 128], bf16, kind="ExternalInput").ap()
    Ww_d = nc.dram_tensor("Ww", [128, NC], bf16, kind="ExternalInput").ap()
    bSV_d = nc.dram_tensor("bSV", [1, D], bf16, kind="ExternalInput").ap()
    # bwm[p, col] = b_w for valid cols, -50 for pad cols: folds the b_w add
    # and the pad-kill (exp(-50+beta) ~ 0) into one per-column bias
    bwm_d = nc.dram_tensor("bwm", [128, P], f32, kind="ExternalInput").ap()
    out_d = nc.dram_tensor("out", [BPC, T, D], bf16, kind="ExternalOutput").ap()

    with tile.TileContext(nc) as tc:
        with (
            tc.tile_pool(name="const", bufs=1) as const,
            tc.tile_pool(name="epre", bufs=3) as eprep,
            tc.tile_pool(name="epre2", bufs=3) as eprep2,
            tc.tile_pool(name="ebig", bufs=1) as ebigp,
            tc.tile_pool(name="soft", bufs=3) as softp,
            tc.tile_pool(name="outp", bufs=3) as outp,
            tc.tile_pool(name="pv", bufs=2, space="PSUM") as pvp,
            tc.tile_pool(name="ps", bufs=2, space="PSUM") as psp,
            tc.tile_pool(name="pbeta", bufs=2, space="PSUM") as pbetap,
            tc.tile_pool(name="pqt", bufs=1, space="PSUM") as pqtp,
            tc.tile_pool(name="pfin", bufs=1, space="PSUM") as pfinp,
        ):
            # ---- force the exp/tanh ACT table load to start at t=0 so it
            # hides under the weight DMAs (the set covers tanh+exp+copy)
            warm = const.tile([1, 2], f32)
            nc.vector.memset(warm[:], 0.0)
            warm2 = const.tile([1, 2], f32)
            nc.scalar.activation(warm2[:], warm[:], mybir.ActivationFunctionType.Tanh)

            # ---- inputs: each queue's pieces ordered by first use ----
            WS_sb = const.tile([128, NC, NC, 128], bf16)  # [p, mc, kc, 128]
            WV_sb = const.tile([128, NC, NC, 128], bf16)  # [p, mc, kc, 128]
            hvT_sb = const.tile([128, BPC, NC, 128], bf16)  # [p, slot, kc, t]
            hsT_sb = const.tile([128, NC, P], bf16)
            bSV_sb = const.tile([1, D], bf16)
            Ww_sb = const.tile([128, NC], bf16)
            bwm_sb = const.tile([128, P], f32)
            hs_sb = const.tile([B0, BPC, D], bf16)

            # each queue's first pieces are exactly the first-granule deps,
            # arriving in consumption order (the tile scheduler's CoreSim
            # models per-queue FIFO completion, so queue order = static
            # schedule order); tail tensors last
            # sync queue: proj_S deps in order
            nc.sync.dma_start(out=hsT_sb[:], in_=hsT_d)
            for mc in range(NC):
                nc.sync.dma_start(out=WS_sb[:, mc, :, :], in_=WS_d[mc])
            nc.sync.dma_start(out=Ww_sb[:], in_=Ww_d)
            # scalar queue: biases + V-side rhs (slot 0 first)
            nc.scalar.dma_start(out=bSV_sb[:], in_=bSV_d)
            for k in range(BPC):
                nc.scalar.dma_start(out=hvT_sb[:, k, :, :], in_=hvT_d[k])
            # gpsimd queue: V-side weights first; tail tensors last
            for mc in range(NC):
                nc.gpsimd.dma_start(out=WV_sb[:, mc, :, :], in_=WV_d[mc])
            nc.gpsimd.dma_start(out=bwm_sb[:], in_=bwm_d)
            nc.gpsimd.dma_start(out=hs_sb[:], in_=hs_d)

            ident = const.tile([128, 128], f32)
            make_identity(nc, ident[:])
            onesP = const.tile([1, P], bf16)
            nc.vector.memset(onesP[:], 1.0)

            VT_sb = const.tile([128, BPC, NC, 128], bf16)
            ST_dup = const.tile([128, NC, P, 2], bf16)

            # ---- helpers -------------------------------------------------
            def proj_S(mc):
                # S chunk: ST'[d, packed(slot, n)] for all slots
                ps_s = psp.tile([128, P], f32, tag="ps")
                for kc in range(NC):
                    nc.tensor.matmul(
                        ps_s[:],
                        WS_sb[:, mc, kc, :],
                        hsT_sb[:, kc, :],
                        start=(kc == 0),
                        stop=False,
                    )
                # + (b_S + b_V) broadcast along packed cols: rank-1 K=1 matmul
                nc.tensor.matmul(
                    ps_s[:],
                    bSV_sb[0:1, mc * 128 : (mc + 1) * 128],
                    onesP[0:1, :],
                    start=False,
                    stop=True,
                )
                # ST_dup[d, mc, p, 2] <- ps_s duplicated over pair axis; for
                # chunk 0 split slot 0 out so the head of the ACT chain isn't
                # gated on the full-P cast
                if mc == 0:
                    nc.vector.tensor_copy(
                        ST_dup[:, mc, 0 : offs[1], :],
                        ps_s[:, 0 : offs[1]]
                        .unsqueeze(2)
                        .broadcast_to([128, bounds[0], 2]),
                    )
                    nc.vector.tensor_copy(
                        ST_dup[:, mc, offs[1] :, :],
                        ps_s[:, offs[1] :]
                        .unsqueeze(2)
                        .broadcast_to([128, P - offs[1], 2]),
                    )
                else:
                    nc.vector.tensor_copy(
                        ST_dup[:, mc, :, :],
                        ps_s[:].unsqueeze(2).broadcast_to([128, P, 2]),
                    )

            def proj_V(mc, k0, k1):
                # V chunk for slots [k0, k1): slots packed in the rhs free
                # dim -> single psum accumulation group
                pv_t = pvp.tile([128, BPC, 128], f32, tag="pv")
                for kc in range(NC):
                    nc.tensor.matmul(
                        pv_t[:, k0:k1, :],
                        WV_sb[:, mc, kc, :],
                        hvT_sb[:, k0:k1, kc, :],
                        start=(kc == 0),
                        stop=(kc == NC - 1),
                    )
                nc.vector.tensor_copy(VT_sb[:, k0:k1, mc, :], pv_t[:, k0:k1, :])

            def ep_add(k, c, ep_slice, n0, n1):
                """e_pre = VT (+bcast over n) + ST' (+bcast over t-pairs)
                for slot k chunk c, rows [n0,n1), into ep_slice."""
                nn = n1 - n0
                nc.vector.tensor_add(
                    ep_slice.rearrange("p n (t two) -> p n t two", two=2),
                    VT_sb[:, k, c, :]
                    .rearrange("p (t two) -> p t two", two=2)
                    .unsqueeze(1)
                    .broadcast_to([128, nn, 64, 2]),
                    ST_dup[:, c, offs[k] + n0 : offs[k] + n1, :]
                    .unsqueeze(2)
                    .broadcast_to([128, nn, 64, 2]),
                )

            def beta_mms(k, c, eb, beta_big, n0, n1):
                for n in range(n0, n1):
                    nc.tensor.matmul(
                        beta_big[:, c, n : n + 1],
                        eb[:, c, n, :],
                        Ww_sb[:, c : c + 1],
                        start=True,
                        stop=True,
                    )

            def add_tanh(k, c, eb):
                """add (DVE 2x) -> tanh (ACT) for one chunk; betas deferred."""
                bk = bounds[k]
                ep = eprep.tile([128, B0, 128], bf16, tag="ep")
                ep_add(k, c, ep[:, 0:bk, :], 0, bk)
                nc.scalar.activation(
                    eb[:, c, :, :], ep[:, 0:bk, :], mybir.ActivationFunctionType.Tanh
                )

            def add_tanh_head(k, c, eb, beta_big):
                """First granule: add/tanh split by n-halves so the ACT chain
                starts on a half-size dependency; betas emitted inline (they
                are the first PE work after projections)."""
                bk = bounds[k]
                h = max(1, bk // 2)
                ep = eprep.tile([128, B0, 128], bf16, tag="ep")
                for n0, n1 in ((0, h), (h, bk)):
                    if n1 <= n0:
                        continue
                    ep_add(k, c, ep[:, n0:n1, :], n0, n1)
                    nc.scalar.activation(
                        eb[:, c, n0:n1, :],
                        ep[:, n0:n1, :],
                        mybir.ActivationFunctionType.Tanh,
                    )

            def granule2(k, c0, eb, beta_big, tail=False):
                """2-chunk granule: two adds, one tanh, betas for both chunks.
                tail=True splits the second chunk's tanh by n-halves so the
                final beta matmuls and softmax start earlier."""
                bk = bounds[k]
                ep = eprep2.tile([128, 2, bounds[1], 128], bf16, tag="ep2")
                ep_add(k, c0, ep[:, 0, 0:bk, :], 0, bk)
                ep_add(k, c0 + 1, ep[:, 1, 0:bk, :], 0, bk)
                if not tail:
                    nc.scalar.activation(
                        eb[:, c0 : c0 + 2, :, :],
                        ep[:, :, 0:bk, :],
                        mybir.ActivationFunctionType.Tanh,
                    )
                    beta_mms(k, c0, eb, beta_big, 0, bk)
                    beta_mms(k, c0 + 1, eb, beta_big, 0, bk)
                else:
                    h = max(1, bk // 2)
                    nc.scalar.activation(
                        eb[:, c0, :, :],
                        ep[:, 0, 0:bk, :],
                        mybir.ActivationFunctionType.Tanh,
                    )
                    beta_mms(k, c0, eb, beta_big, 0, bk)
                    nc.scalar.activation(
                        eb[:, c0 + 1, 0:h, :],
                        ep[:, 1, 0:h, :],
                        mybir.ActivationFunctionType.Tanh,
                    )
                    beta_mms(k, c0 + 1, eb, beta_big, 0, h)
                    nc.scalar.activation(
                        eb[:, c0 + 1, h:bk, :],
                        ep[:, 1, h:bk, :],
                        mybir.ActivationFunctionType.Tanh,
                    )
                    beta_mms(k, c0 + 1, eb, beta_big, h, bk)

            # per-slot softmax state: chunk-pair sums pulled out of PSUM as
            # soon as each half of beta is complete
            s2_tiles = {}

            def beta_pair_copy(k, half, beta_big):
                bk = bounds[k]
                if half == 0:
                    s2_tiles[k] = softp.tile([128, 2, 2, B0], f32, tag="s2c", name=f"s2c{k}")
                nc.vector.tensor_copy(
                    s2_tiles[k][:, half, :, 0:bk],
                    beta_big[:, 2 * half : 2 * half + 2, :],
                )

            def softmax_final(k):
                bk = bounds[k]
                s2c = s2_tiles[k]
                s2 = softp.tile([128, 2, B0], f32, tag="s2")
                nc.vector.tensor_add(
                    s2[:, :, 0:bk], s2c[:, 0, :, 0:bk], s2c[:, 1, :, 0:bk]
                )
                qa = softp.tile([128, B0], f32, tag="qa")
                # qa = (c0+c2) + (c1+c3) + bwm; bwm = b_w on valid cols,
                # -50 on pad cols so exp kills them (no mask mult needed)
                qs = softp.tile([128, B0], f32, tag="qs")
                nc.vector.tensor_add(qs[:, 0:bk], s2[:, 0, 0:bk], s2[:, 1, 0:bk])
                nc.vector.tensor_add(
                    qa[:, 0:bk], qs[:, 0:bk], bwm_sb[:, offs[k] : offs[k] + bk]
                )
                # exp directly on qa: pad positions are killed in the
                # numerator by the zeroed h_s rows and in the denominator by
                # the -50 bias
                t1 = softp.tile([128, B0], f32, tag="t1")
                nc.scalar.activation(
                    t1[:, 0:bk], qa[:, 0:bk], mybir.ActivationFunctionType.Exp
                )
                Qs = softp.tile([128, 1], f32, tag="Z1")
                nc.vector.tensor_reduce(
                    Qs[:], t1[:, 0:bk], mybir.AxisListType.X, mybir.AluOpType.add
                )
                recip = softp.tile([128, 1], f32, tag="recip")
                nc.vector.reciprocal(recip[:], Qs[:])
                # ---- out[k] = (t1 @ h_s_masked[k]) * recip ----
                qT_ps = pqtp.tile([B0, 128], f32, tag="qt")
                nc.tensor.transpose(qT_ps[0:bk, :], t1[:, 0:bk], ident[:])
                qT = softp.tile([B0, 128], bf16, tag="qTs")
                nc.vector.tensor_copy(qT[0:bk, :], qT_ps[0:bk, :])
                out_ps = pfinp.tile([128, D], f32, tag="out")
                out_sb = outp.tile([128, D], bf16, tag="osb")
                if k == BPC - 1:
                    # tail: split matmul/scale by D-halves so copy and DMA
                    # overlap the second half's matmul
                    for hf in range(2):
                        cs2 = slice(hf * (D // 2), (hf + 1) * (D // 2))
                        nc.tensor.matmul(
                            out_ps[:, cs2],
                            qT[0:bk, :],
                            hs_sb[0:bk, k, cs2],
                            start=True,
                            stop=True,
                        )
                        nc.vector.tensor_scalar_mul(
                            out_sb[:, cs2], out_ps[:, cs2], recip[:]
                        )
                        nc.sync.dma_start(out=out_d[k][:, cs2], in_=out_sb[:, cs2])
                else:
                    nc.tensor.matmul(
                        out_ps[:], qT[0:bk, :], hs_sb[0:bk, k, :], start=True, stop=True
                    )
                    nc.vector.tensor_scalar_mul(out_sb[:], out_ps[:], recip[:])
                    nc.sync.dma_start(out=out_d[k], in_=out_sb[:])

            # ---- slot 0 interleaved with projections; beta matmuls are
            # emitted one chunk late so the PE queue never blocks a
            # projection behind tanh-gated work ----
            slot_tiles = {}

            def alloc_slot(k):
                slot_tiles[k] = (
                    ebigp.tile(
                        [128, NC, bounds[k], 128], bf16, tag=f"e{k}", name=f"eb{k}"
                    ),
                    pbetap.tile(
                        [128, NC, bounds[k]], f32, tag="beta", name=f"bb{k}"
                    ),
                )

            # slot-0 V projections are per-chunk (only need hvT0+WV_mc) so
            # the tanh cadence never waits for the other slots' hvT pieces;
            # slots 1-3 V projections are batched and deferred below
            for mc in range(NC):
                proj_S(mc)
                proj_V(mc, 0, 1)
                if mc == 0:
                    alloc_slot(0)
                    add_tanh_head(0, 0, *slot_tiles[0])
                else:
                    beta_mms(0, mc - 1, *slot_tiles[0], 0, bounds[0])
                    if mc == 2:
                        beta_pair_copy(0, 0, slot_tiles[0][1])
                    add_tanh(0, mc, slot_tiles[0][0])
            for mc in range(NC):
                proj_V(mc, 1, BPC)
            beta_mms(0, NC - 1, *slot_tiles[0], 0, bounds[0])
            beta_pair_copy(0, 1, slot_tiles[0][1])

            # ---- remaining slots, softmax/final pipelined one slot late ----
            for k in range(1, BPC):
                alloc_slot(k)
                eb, beta_big = slot_tiles[k]
                last = k == BPC - 1
                if not last:
                    granule2(k, 0, eb, beta_big)
                    beta_pair_copy(k, 0, beta_big)
                    softmax_final(k - 1)
                    granule2(k, 2, eb, beta_big)
                    beta_pair_copy(k, 1, beta_big)
                else:
                    # last slot: softmax first so its DVE ops don't sit
                    # between this slot's adds (the end is the critical tail)
                    softmax_final(k - 1)
                    granule2(k, 0, eb, beta_big)
                    beta_pair_copy(k, 0, beta_big)
                    granule2(k, 2, eb, beta_big, tail=True)
                    beta_pair_copy(k, 1, beta_big)
            softmax_final(BPC - 1)

    nc.compile()
    return nc


def _get_nc(bounds):
    key = tuple(bounds)
    if key not in _CACHE:
        _CACHE[key] = _build(list(bounds))
    return _CACHE[key]


def _plan(lengths):
    """Sort batches by length desc; slot k on core c <- sorted rank 8k+c.
    Returns (order, bounds)."""
    lengths = np.asarray(lengths).reshape(-1)
    order = np.argsort(-lengths, kind="stable")
    bounds = [int(lengths[order[NCORES * k]]) for k in range(BPC)]
    return order, bounds


def _make_in_maps(order, bounds, h_s, h_v, lengths, W_S, b_S, W_V, b_V, W_w, b_w):
    f32 = np.float32
    h_s = np.asarray(h_s, dtype=f32)
    h_v = np.asarray(h_v, dtype=f32)
    lengths = np.asarray(lengths).reshape(-1)
    offs = np.concatenate([[0], np.cumsum(bounds)]).astype(int)
    P = int(offs[-1])
    B0 = bounds[0]

    # weights, chunked + cast once (shared across cores); mc outermost
    WS = np.ascontiguousarray(
        np.asarray(W_S, f32).reshape(NC, 128, NC, 128).transpose(2, 1, 0, 3)
    )  # [mc, p, kc, 128]
    WV = np.ascontiguousarray(
        np.asarray(W_V, f32).reshape(NC, 128, NC, 128).transpose(2, 1, 0, 3)
    )
    Ww = np.ascontiguousarray(np.asarray(W_w, f32).reshape(NC, 128).T)
    bSV = (np.asarray(b_S, f32) + np.asarray(b_V, f32)).reshape(1, D)
    bw_val = f32(np.asarray(b_w).reshape(-1)[0])

    try:
        import ml_dtypes

        bf16 = ml_dtypes.bfloat16
    except ImportError:
        import jax.numpy as jnp

        bf16 = jnp.bfloat16

    def to_bf16(x):
        return np.asarray(x, dtype=bf16)

    WS_b = to_bf16(WS)
    WV_b = to_bf16(WV)
    Ww_b = to_bf16(Ww)
    bSV_b = to_bf16(bSV)

    in_maps = []
    for core in range(NCORES):
        batches = [int(order[NCORES * k + core]) for k in range(BPC)]
        hv_c = h_v[batches]  # (BPC, T, D)
        hvT = np.ascontiguousarray(
            hv_c.reshape(BPC, T, NC, 128).transpose(0, 3, 2, 1)
        )  # (slot, 128p, kc, t)
        hsT = np.zeros((128, NC, P), dtype=f32)
        hs_r = np.zeros((B0, BPC, D), dtype=f32)  # (n, slot, D), masked rows 0
        bwm = np.full((128, P), -50.0, dtype=f32)
        for k, b in enumerate(batches):
            L = int(lengths[b])
            bk = bounds[k]
            Lk = min(L, bk)
            hk = h_s[b, :Lk]  # (Lk, D)
            hsT[:, :, offs[k] : offs[k] + Lk] = hk.reshape(Lk, NC, 128).transpose(
                2, 1, 0
            )
            hs_r[:Lk, k, :] = hk
            bwm[:, offs[k] : offs[k] + Lk] = bw_val
        in_maps.append(
            {
                "hvT": to_bf16(hvT),
                "hsT": to_bf16(hsT),
                "hs": to_bf16(hs_r),
                "WS": WS_b,
                "WV": WV_b,
                "Ww": Ww_b,
                "bSV": bSV_b,
                "bwm": bwm,
            }
        )
    return in_maps


def run(inputs: dict, trace: bool = False):
    """Run on 8 NeuronCores; returns (output, BassKernelResults)."""
    from concourse import bass_utils

    order, bounds = _plan(inputs["lengths"])
    nc = _get_nc(bounds)
    in_maps = _make_in_maps(order, bounds, **inputs)
    res = bass_utils.run_bass_kernel_spmd(
        nc, in_maps, core_ids=list(range(NCORES)), trace=trace
    )
    full = np.zeros((B, T, D), dtype=np.float32)
    for core in range(NCORES):
        o = np.asarray(res.results[core]["out"], dtype=np.float32)
        for k in range(BPC):
            full[int(order[NCORES * k + core])] = o[k]
    return full, res


def kernel(**inputs) -> np.ndarray:
    out, _ = run(inputs, trace=False)
    return out


# revision 15
# speedup vs baseline: 1.1413x; 1.1413x over previous
"""Trainium2 Bass kernel for InteractorwoLSTM additive attention.

out[b,t,:] = alpha[b,t,:] @ h_s[b]  with
  beta[b,t,n] = W_w . tanh(h_s[b,n]@W_S + b_S + h_v[b,t]@W_V + b_V) + b_w
  alpha = masked-softmax(beta) per reference semantics.

v4 design: length-truncated slots + JIT DMA + tight ramp/tail.

Positions n >= lengths[b] never influence the output (the reference's
masked-softmax renormalization cancels them), so the tanh/add/beta work
for those positions is skipped. lengths are known on the host before
compile, so the program is built for the actual length profile:
batches sorted by length desc; slot k on core c holds sorted rank
8k+c; slot k's n-bound = L(rank 8k) (SPMD shares one program). For the
seed-0 data: bounds [30,17,12,8] -> 67 packed n-columns vs 120
(~44% less work on ACT -- the bottleneck engine at 1 elem/cycle/lane).

v4 over v3 (v3 = 59.8us, ACT busy 34.5us, ramp-to-first-tanh 16us,
tail 9us):
  - weight/input DRAM layouts are chunk-outermost so each DMA piece is
    a contiguous 1KB-per-partition run; pieces are ordered/queued so
    the first granule's deps (WS0/hsT/WV0/hvT0) land first.
  - slot-0 beta matmuls are emitted one chunk late so the PE queue
    never blocks projections behind tanh-gated work.
  - softmax chunk-sums read beta PSUM in c01/c23 halves as soon as
    each half is done -> the tail only carries the c23 copy.
  - last granule and final matmul/copy split for a shorter tail.
  - dummy tanh at the top forces the ACT table load (~2.7us) under the
    initial DMAs.
"""

import numpy as np

B, T, N = 32, 128, 30
D = 512
NCORES = 8
BPC = B // NCORES  # batch slots per core
NC = D // 128  # 4 chunks of 128 along D

_CACHE = {}


def _build(bounds):
    import concourse.bacc as bacc
    import concourse.tile as tile
    from concourse import mybir
    from concourse.masks import make_identity

    f32 = mybir.dt.float32
    bf16 = mybir.dt.bfloat16

    offs = [0]
    for b in bounds:
        offs.append(offs[-1] + b)
    P = offs[-1]
    B0 = bounds[0]

    nc = bacc.Bacc(
        "TRN2",
        target_bir_lowering=False,
        debug=False,
        enable_asserts=True,
        num_devices=NCORES,
    )

    # ---- DRAM I/O (host-prepped layouts, chunk-outermost for DMA) ----
    hvT_d = nc.dram_tensor("hvT", [BPC, 128, NC, 128], bf16, kind="ExternalInput").ap()
    hsT_d = nc.dram_tensor("hsT", [128, NC, P], bf16, kind="ExternalInput").ap()
    hs_d = nc.dram_tensor("hs", [B0, BPC, D], bf16, kind="ExternalInput").ap()
    WS_d = nc.dram_tensor("WS", [NC, 128, NC, 128], bf16, kind="ExternalInput").ap()
    WV_d = nc.dram_tensor("WV", [NC, 128, NC, 128], bf16, kind="ExternalInput").ap()
    Ww_d = nc.dram_tensor("Ww", [128, NC], bf16, kind="ExternalInput").ap()
    bSV_d = nc.dram_tensor("bSV", [1, D], bf16, kind="ExternalInput").ap()
    # bwm[p, col] = b_w for valid cols, -50 for pad cols: folds the b_w add
    # and the pad-kill (exp(-50+beta) ~ 0) into one per-column bias
    bwm_d = nc.dram_tensor("bwm", [128, P], f32, kind="ExternalInput").ap()
    out_d = nc.dram_tensor("out", [BPC, T, D], bf16, kind="ExternalOutput").ap()

    with tile.TileContext(nc) as tc:
        with (
            tc.tile_pool(name="const", bufs=1) as const,
            tc.tile_pool(name="epre", bufs=3) as eprep,
            tc.tile_pool(name="epre2", bufs=3) as eprep2,
            tc.tile_pool(name="ebig", bufs=1) as ebigp,
            tc.tile_pool(name="soft", bufs=3) as softp,
            tc.tile_pool(name="outp", bufs=3) as outp,
            tc.tile_pool(name="pv", bufs=2, space="PSUM") as pvp,
            tc.tile_pool(name="ps", bufs=2, space="PSUM") as psp,
            tc.tile_pool(name="pbeta", bufs=2, space="PSUM") as pbetap,
            tc.tile_pool(name="pqt", bufs=1, space="PSUM") as pqtp,
            tc.tile_pool(name="pfin", bufs=1, space="PSUM") as pfinp,
        ):
            # ---- force the exp/tanh ACT table load to start at t=0 so it
            # hides under the weight DMAs (the set covers tanh+exp+copy)
            warm = const.tile([1, 2], f32)
            nc.vector.memset(warm[:], 0.0)
            warm2 = const.tile([1, 2], f32)
            nc.scalar.activation(warm2[:], warm[:], mybir.ActivationFunctionType.Tanh)

            # ---- inputs: each queue's pieces ordered by first use ----
            WS_sb = const.tile([128, NC, NC, 128], bf16)  # [p, mc, kc, 128]
            WV_sb = const.tile([128, NC, NC, 128], bf16)  # [p, mc, kc, 128]
            hvT_sb = const.tile([128, BPC, NC, 128], bf16)  # [p, slot, kc, t]
            hsT_sb = const.tile([128, NC, P], bf16)
            bSV_sb = const.tile([1, D], bf16)
            Ww_sb = const.tile([128, NC], bf16)
            bwm_sb = const.tile([128, P], f32)
            hs_sb = const.tile([B0, BPC, D], bf16)

            # DMA pieces are spread over the 3 issue queues (sync/scalar/
            # gpsimd) and EMITTED in global arrival order: the tile
            # scheduler's CoreSim serializes all DMA transfers through one
            # global resource in emission-priority order, so emission order
            # here IS the arrival order the static schedule is built for.
            # Per-queue FIFO order (what hardware sees) is the subsequence
            # per engine: sync=[hsT,WS0..3,Ww], scalar=[bSV,hvT0..3],
            # gpsimd=[WV0..3,bwm,hs] -- first-granule deps lead each queue.
            nc.scalar.dma_start(out=bSV_sb[:], in_=bSV_d)
            nc.sync.dma_start(out=hsT_sb[:], in_=hsT_d)
            nc.scalar.dma_start(out=hvT_sb[:, 0, :, :], in_=hvT_d[0])
            nc.gpsimd.dma_start(out=WV_sb[:, 0, :, :], in_=WV_d[0])
            nc.sync.dma_start(out=WS_sb[:, 0, :, :], in_=WS_d[0])
            for mc in range(1, NC):
                nc.sync.dma_start(out=WS_sb[:, mc, :, :], in_=WS_d[mc])
                nc.gpsimd.dma_start(out=WV_sb[:, mc, :, :], in_=WV_d[mc])
                nc.scalar.dma_start(out=hvT_sb[:, mc, :, :], in_=hvT_d[mc])
            nc.sync.dma_start(out=Ww_sb[:], in_=Ww_d)
            nc.gpsimd.dma_start(out=bwm_sb[:], in_=bwm_d)
            nc.gpsimd.dma_start(out=hs_sb[:], in_=hs_d)

            ident = const.tile([128, 128], f32)
            make_identity(nc, ident[:])
            onesP = const.tile([1, P], bf16)
            nc.vector.memset(onesP[:], 1.0)

            VT_sb = const.tile([128, BPC, NC, 128], bf16)
            ST_dup = const.tile([128, NC, P, 2], bf16)

            # ---- helpers -------------------------------------------------
            def proj_S(mc):
                # S chunk: ST'[d, packed(slot, n)] for all slots
                ps_s = psp.tile([128, P], f32, tag="ps")
                for kc in range(NC):
                    nc.tensor.matmul(
                        ps_s[:],
                        WS_sb[:, mc, kc, :],
                        hsT_sb[:, kc, :],
                        start=(kc == 0),
                        stop=False,
                    )
                # + (b_S + b_V) broadcast along packed cols: rank-1 K=1 matmul
                nc.tensor.matmul(
                    ps_s[:],
                    bSV_sb[0:1, mc * 128 : (mc + 1) * 128],
                    onesP[0:1, :],
                    start=False,
                    stop=True,
                )
                # ST_dup[d, mc, p, 2] <- ps_s duplicated over pair axis; for
                # chunk 0 split slot 0 out so the head of the ACT chain isn't
                # gated on the full-P cast
                if mc == 0:
                    nc.vector.tensor_copy(
                        ST_dup[:, mc, 0 : offs[1], :],
                        ps_s[:, 0 : offs[1]]
                        .unsqueeze(2)
                        .broadcast_to([128, bounds[0], 2]),
                    )
                    nc.vector.tensor_copy(
                        ST_dup[:, mc, offs[1] :, :],
                        ps_s[:, offs[1] :]
                        .unsqueeze(2)
                        .broadcast_to([128, P - offs[1], 2]),
                    )
                else:
                    nc.vector.tensor_copy(
                        ST_dup[:, mc, :, :],
                        ps_s[:].unsqueeze(2).broadcast_to([128, P, 2]),
                    )

            def proj_V(mc, k0, k1):
                # V chunk for slots [k0, k1): slots packed in the rhs free
                # dim -> single psum accumulation group
                pv_t = pvp.tile([128, BPC, 128], f32, tag="pv")
                for kc in range(NC):
                    nc.tensor.matmul(
                        pv_t[:, k0:k1, :],
                        WV_sb[:, mc, kc, :],
                        hvT_sb[:, k0:k1, kc, :],
                        start=(kc == 0),
                        stop=(kc == NC - 1),
                    )
                nc.vector.tensor_copy(VT_sb[:, k0:k1, mc, :], pv_t[:, k0:k1, :])

            def ep_add(k, c, ep_slice, n0, n1):
                """e_pre = VT (+bcast over n) + ST' (+bcast over t-pairs)
                for slot k chunk c, rows [n0,n1), into ep_slice."""
                nn = n1 - n0
                nc.vector.tensor_add(
                    ep_slice.rearrange("p n (t two) -> p n t two", two=2),
                    VT_sb[:, k, c, :]
                    .rearrange("p (t two) -> p t two", two=2)
                    .unsqueeze(1)
                    .broadcast_to([128, nn, 64, 2]),
                    ST_dup[:, c, offs[k] + n0 : offs[k] + n1, :]
                    .unsqueeze(2)
                    .broadcast_to([128, nn, 64, 2]),
                )

            def beta_mms(k, c, eb, beta_big, n0, n1):
                for n in range(n0, n1):
                    nc.tensor.matmul(
                        beta_big[:, c, n : n + 1],
                        eb[:, c, n, :],
                        Ww_sb[:, c : c + 1],
                        start=True,
                        stop=True,
                    )

            def add_tanh(k, c, eb):
                """add (DVE 2x) -> tanh (ACT) for one chunk; betas deferred."""
                bk = bounds[k]
                ep = eprep.tile([128, B0, 128], bf16, tag="ep")
                ep_add(k, c, ep[:, 0:bk, :], 0, bk)
                nc.scalar.activation(
                    eb[:, c, :, :], ep[:, 0:bk, :], mybir.ActivationFunctionType.Tanh
                )

            def add_tanh_head(k, c, eb, beta_big):
                """First granule: add/tanh split by n-halves so the ACT chain
                starts on a half-size dependency; betas emitted inline (they
                are the first PE work after projections)."""
                bk = bounds[k]
                h = max(1, bk // 2)
                ep = eprep.tile([128, B0, 128], bf16, tag="ep")
                for n0, n1 in ((0, h), (h, bk)):
                    if n1 <= n0:
                        continue
                    ep_add(k, c, ep[:, n0:n1, :], n0, n1)
                    nc.scalar.activation(
                        eb[:, c, n0:n1, :],
                        ep[:, n0:n1, :],
                        mybir.ActivationFunctionType.Tanh,
                    )

            def granule2(k, c0, eb, beta_big, tail=False):
                """2-chunk granule: two adds, one tanh, betas for both chunks.
                tail=True splits the second chunk's tanh by n-halves so the
                final beta matmuls and softmax start earlier."""
                bk = bounds[k]
                ep = eprep2.tile([128, 2, bounds[1], 128], bf16, tag="ep2")
                ep_add(k, c0, ep[:, 0, 0:bk, :], 0, bk)
                ep_add(k, c0 + 1, ep[:, 1, 0:bk, :], 0, bk)
                if not tail:
                    nc.scalar.activation(
                        eb[:, c0 : c0 + 2, :, :],
                        ep[:, :, 0:bk, :],
                        mybir.ActivationFunctionType.Tanh,
                    )
                    beta_mms(k, c0, eb, beta_big, 0, bk)
                    beta_mms(k, c0 + 1, eb, beta_big, 0, bk)
                else:
                    h = max(1, bk // 2)
                    nc.scalar.activation(
                        eb[:, c0, :, :],
                        ep[:, 0, 0:bk, :],
                        mybir.ActivationFunctionType.Tanh,
                    )
                    beta_mms(k, c0, eb, beta_big, 0, bk)
                    nc.scalar.activation(
                        eb[:, c0 + 1, 0:h, :],
                        ep[:, 1, 0:h, :],
                        mybir.ActivationFunctionType.Tanh,
                    )
                    beta_mms(k, c0 + 1, eb, beta_big, 0, h)
                    nc.scalar.activation(
                        eb[:, c0 + 1, h:bk, :],
                        ep[:, 1, h:bk, :],
                        mybir.ActivationFunctionType.Tanh,
                    )
                    beta_mms(k, c0 + 1, eb, beta_big, h, bk)

            # per-slot softmax state: chunk-pair sums pulled out of PSUM as
            # soon as each half of beta is complete
            s2_tiles = {}

            def beta_pair_copy(k, half, beta_big):
                bk = bounds[k]
                if half == 0:
                    s2_tiles[k] = softp.tile([128, 2, 2, B0], f32, tag="s2c", name=f"s2c{k}")
                nc.vector.tensor_copy(
                    s2_tiles[k][:, half, :, 0:bk],
                    beta_big[:, 2 * half : 2 * half + 2, :],
                )

            def softmax_final(k):
                bk = bounds[k]
                s2c = s2_tiles[k]
                s2 = softp.tile([128, 2, B0], f32, tag="s2")
                nc.vector.tensor_add(
                    s2[:, :, 0:bk], s2c[:, 0, :, 0:bk], s2c[:, 1, :, 0:bk]
                )
                qa = softp.tile([128, B0], f32, tag="qa")
                # qa = (c0+c2) + (c1+c3) + bwm; bwm = b_w on valid cols,
                # -50 on pad cols so exp kills them (no mask mult needed)
                qs = softp.tile([128, B0], f32, tag="qs")
                nc.vector.tensor_add(qs[:, 0:bk], s2[:, 0, 0:bk], s2[:, 1, 0:bk])
                nc.vector.tensor_add(
                    qa[:, 0:bk], qs[:, 0:bk], bwm_sb[:, offs[k] : offs[k] + bk]
                )
                # exp directly on qa: pad positions are killed in the
                # numerator by the zeroed h_s rows and in the denominator by
                # the -50 bias
                t1 = softp.tile([128, B0], f32, tag="t1")
                nc.scalar.activation(
                    t1[:, 0:bk], qa[:, 0:bk], mybir.ActivationFunctionType.Exp
                )
                Qs = softp.tile([128, 1], f32, tag="Z1")
                nc.vector.tensor_reduce(
                    Qs[:], t1[:, 0:bk], mybir.AxisListType.X, mybir.AluOpType.add
                )
                recip = softp.tile([128, 1], f32, tag="recip")
                nc.vector.reciprocal(recip[:], Qs[:])
                # ---- out[k] = (t1 @ h_s_masked[k]) * recip ----
                qT_ps = pqtp.tile([B0, 128], f32, tag="qt")
                nc.tensor.transpose(qT_ps[0:bk, :], t1[:, 0:bk], ident[:])
                qT = softp.tile([B0, 128], bf16, tag="qTs")
                nc.vector.tensor_copy(qT[0:bk, :], qT_ps[0:bk, :])
                out_ps = pfinp.tile([128, D], f32, tag="out")
                out_sb = outp.tile([128, D], bf16, tag="osb")
                if k == BPC - 1:
                    # tail: split matmul/scale by D-halves so copy and DMA
                    # overlap the second half's matmul
                    for hf in range(2):
                        cs2 = slice(hf * (D // 2), (hf + 1) * (D // 2))
                        nc.tensor.matmul(
                            out_ps[:, cs2],
                            qT[0:bk, :],
                            hs_sb[0:bk, k, cs2],
                            start=True,
                            stop=True,
                        )
                        nc.vector.tensor_scalar_mul(
                            out_sb[:, cs2], out_ps[:, cs2], recip[:]
                        )
                        nc.sync.dma_start(out=out_d[k][:, cs2], in_=out_sb[:, cs2])
                else:
                    nc.tensor.matmul(
                        out_ps[:], qT[0:bk, :], hs_sb[0:bk, k, :], start=True, stop=True
                    )
                    nc.vector.tensor_scalar_mul(out_sb[:], out_ps[:], recip[:])
                    nc.sync.dma_start(out=out_d[k], in_=out_sb[:])

            # ---- slot 0 interleaved with projections; beta matmuls are
            # emitted one chunk late so the PE queue never blocks a
            # projection behind tanh-gated work ----
            slot_tiles = {}

            def alloc_slot(k):
                slot_tiles[k] = (
                    ebigp.tile(
                        [128, NC, bounds[k], 128], bf16, tag=f"e{k}", name=f"eb{k}"
                    ),
                    pbetap.tile(
                        [128, NC, bounds[k]], f32, tag="beta", name=f"bb{k}"
                    ),
                )

            # slot-0 V projections are per-chunk (only need hvT0+WV_mc) so
            # the tanh cadence never waits for the other slots' hvT pieces;
            # slots 1-3 V projections are batched and deferred below
            for mc in range(NC):
                proj_S(mc)
                proj_V(mc, 0, 1)
                if mc == 0:
                    alloc_slot(0)
                    add_tanh_head(0, 0, *slot_tiles[0])
                else:
                    beta_mms(0, mc - 1, *slot_tiles[0], 0, bounds[0])
                    if mc == 2:
                        beta_pair_copy(0, 0, slot_tiles[0][1])
                    add_tanh(0, mc, slot_tiles[0][0])
            for mc in range(NC):
                proj_V(mc, 1, BPC)
            beta_mms(0, NC - 1, *slot_tiles[0], 0, bounds[0])
            beta_pair_copy(0, 1, slot_tiles[0][1])

            # ---- remaining slots, softmax/final pipelined one slot late ----
            for k in range(1, BPC):
                alloc_slot(k)
                eb, beta_big = slot_tiles[k]
                last = k == BPC - 1
                if not last:
                    granule2(k, 0, eb, beta_big)
                    beta_pair_copy(k, 0, beta_big)
                    softmax_final(k - 1)
                    granule2(k, 2, eb, beta_big)
                    beta_pair_copy(k, 1, beta_big)
                else:
                    # last slot: softmax first so its DVE ops don't sit
                    # between this slot's adds (the end is the critical tail)
                    softmax_final(k - 1)
                    granule2(k, 0, eb, beta_big)
                    beta_pair_copy(k, 0, beta_big)
                    granule2(k, 2, eb, beta_big, tail=True)
                    beta_pair_copy(k, 1, beta_big)
            softmax_final(BPC - 1)

    nc.compile()
    return nc


def _get_nc(bounds):
    key = tuple(bounds)
    if key not in _CACHE:
        _CACHE[key] = _build(list(bounds))
    return _CACHE[key]


def _plan(lengths):
    """Sort batches by length desc; slot k on core c <- sorted rank 8k+c.
    Returns (order, bounds)."""
    lengths = np.asarray(lengths).reshape(-1)
    order = np.argsort(-lengths, kind="stable")
    bounds = [int(lengths[order[NCORES * k]]) for k in range(BPC)]
    return order, bounds


def _make_in_maps(order, bounds, h_s, h_v, lengths, W_S, b_S, W_V, b_V, W_w, b_w):
    f32 = np.float32
    h_s = np.asarray(h_s, dtype=f32)
    h_v = np.asarray(h_v, dtype=f32)
    lengths = np.asarray(lengths).reshape(-1)
    offs = np.concatenate([[0], np.cumsum(bounds)]).astype(int)
    P = int(offs[-1])
    B0 = bounds[0]

    # weights, chunked + cast once (shared across cores); mc outermost
    WS = np.ascontiguousarray(
        np.asarray(W_S, f32).reshape(NC, 128, NC, 128).transpose(2, 1, 0, 3)
    )  # [mc, p, kc, 128]
    WV = np.ascontiguousarray(
        np.asarray(W_V, f32).reshape(NC, 128, NC, 128).transpose(2, 1, 0, 3)
    )
    Ww = np.ascontiguousarray(np.asarray(W_w, f32).reshape(NC, 128).T)
    bSV = (np.asarray(b_S, f32) + np.asarray(b_V, f32)).reshape(1, D)
    bw_val = f32(np.asarray(b_w).reshape(-1)[0])

    try:
        import ml_dtypes

        bf16 = ml_dtypes.bfloat16
    except ImportError:
        import jax.numpy as jnp

        bf16 = jnp.bfloat16

    def to_bf16(x):
        return np.asarray(x, dtype=bf16)

    WS_b = to_bf16(WS)
    WV_b = to_bf16(WV)
    Ww_b = to_bf16(Ww)
    bSV_b = to_bf16(bSV)

    in_maps = []
    for core in range(NCORES):
        batches = [int(order[NCORES * k + core]) for k in range(BPC)]
        hv_c = h_v[batches]  # (BPC, T, D)
        hvT = np.ascontiguousarray(
            hv_c.reshape(BPC, T, NC, 128).transpose(0, 3, 2, 1)
        )  # (slot, 128p, kc, t)
        hsT = np.zeros((128, NC, P), dtype=f32)
        hs_r = np.zeros((B0, BPC, D), dtype=f32)  # (n, slot, D), masked rows 0
        bwm = np.full((128, P), -50.0, dtype=f32)
        for k, b in enumerate(batches):
            L = int(lengths[b])
            bk = bounds[k]
            Lk = min(L, bk)
            hk = h_s[b, :Lk]  # (Lk, D)
            hsT[:, :, offs[k] : offs[k] + Lk] = hk.reshape(Lk, NC, 128).transpose(
                2, 1, 0
            )
            hs_r[:Lk, k, :] = hk
            bwm[:, offs[k] : offs[k] + Lk] = bw_val
        in_maps.append(
            {
                "hvT": to_bf16(hvT),
                "hsT": to_bf16(hsT),
                "hs": to_bf16(hs_r),
                "WS": WS_b,
                "WV": WV_b,
                "Ww": Ww_b,
                "bSV": bSV_b,
                "bwm": bwm,
            }
        )
    return in_maps


def run(inputs: dict, trace: bool = False):
    """Run on 8 NeuronCores; returns (output, BassKernelResults)."""
    from concourse import bass_utils

    order, bounds = _plan(inputs["lengths"])
    nc = _get_nc(bounds)
    in_maps = _make_in_maps(order, bounds, **inputs)
    res = bass_utils.run_bass_kernel_spmd(
        nc, in_maps, core_ids=list(range(NCORES)), trace=trace
    )
    full = np.zeros((B, T, D), dtype=np.float32)
    for core in range(NCORES):
        o = np.asarray(res.results[core]["out"], dtype=np.float32)
        for k in range(BPC):
            full[int(order[NCORES * k + core])] = o[k]
    return full, res


def kernel(**inputs) -> np.ndarray:
    out, _ = run(inputs, trace=False)
    return out
